# revision 3
# baseline (speedup 1.0000x reference)
"""Multi-head causal attention (B=2, T=2048, H=16, D=64, C=1024) on 8 trn2 cores.

Sharding: tensor-parallel over heads. Each core owns 2 heads (both batches):
  - computes Q^T/K^T/V^T for its heads over all 4096 tokens
  - causal attention in transposed orientation (S^T[k,q]) so no P transpose
  - partial output projection outT_partial[c, t] = Wo_slice^T @ O^T
Host sums the 8 partials (the "all-reduce"), adds bias, transposes back.

All matmuls run as float32r (fp32 storage, full PE rate for moving dim>=256).
"""

import sys

sys.path.insert(0, "/opt/trn_rl_repo")
sys.path.insert(0, "/root/problem")

import numpy as np

import concourse.bacc as bacc
import concourse.mybir as mybir
import concourse.tile as tile
from concourse.bass_utils import run_bass_kernel_spmd
from concourse.masks import make_identity

B, T, C = 2, 2048, 1024
H, D = 16, 64
NT = B * T  # 4096 flattened tokens
N_CORES = 8
HPC = H // N_CORES  # 2 heads per core
FPC = HPC * D  # 128 features per core
CT = C // 128  # 8 contraction tiles for projections
TBLK = 512  # token block
NTB = NT // TBLK  # 8 token blocks
QB = T // TBLK  # 4 query blocks per batch
KT = T // 128  # 16 key tiles per batch

F32 = mybir.dt.float32
F32R = mybir.dt.float32r


def r32(ap):
    return ap.bitcast(F32R)


def build_program():
    nc = bacc.Bacc("TRN2", target_bir_lowering=False, debug=False)

    xt_d = nc.declare_dram_parameter("xt", [C, NT], F32, isOutput=False)
    wq_d = nc.declare_dram_parameter("wq", [C, FPC], F32, isOutput=False)
    wk_d = nc.declare_dram_parameter("wk", [C, FPC], F32, isOutput=False)
    wv_d = nc.declare_dram_parameter("wv", [C, FPC], F32, isOutput=False)
    wo_d = nc.declare_dram_parameter("wo", [FPC, C], F32, isOutput=False)
    out_d = nc.declare_dram_parameter("outT", [C, NT], F32, isOutput=True)

    with tile.TileContext(nc) as tc:
        with (
            tc.tile_pool(name="slabs", bufs=1) as slabs,
            tc.tile_pool(name="xtp", bufs=16) as xtp,
            tc.tile_pool(name="work", bufs=6) as work,
            tc.tile_pool(name="outp", bufs=4) as outp,
            tc.tile_pool(name="psA", bufs=6, space="PSUM") as psA,
            tc.tile_pool(name="psO", bufs=2, space="PSUM") as psO,
        ):
            # ---- persistent slabs
            qT = slabs.tile([128, NT], F32R, tag="qT")  # [2h*64d, t]
            kT = slabs.tile([128, NT], F32R, tag="kT")
            # V natural layout + ones cols: per (ktile_global, h): [128k, 128]
            # h=0: cols 0:64 = V, 64:128 = ones   (O rows 0:64, rowsum 64:128)
            # h=1: cols 0:64 = ones, 64:128 = V   (rowsum rows 0:64, O 64:128)
            vN = slabs.tile([128, NTB * 4, HPC, 128], F32R, tag="vN")
            oN = slabs.tile([128, NT], F32R, tag="oN")  # normalized O^T
            wq_s = slabs.tile([128, CT, FPC], F32R, tag="wq")
            wk_s = slabs.tile([128, CT, FPC], F32R, tag="wk")
            wv_s = slabs.tile([128, CT, FPC], F32R, tag="wv")
            wo_s = slabs.tile([128, C], F32R, tag="wo")  # [f, c]
            ident = slabs.tile([128, 128], F32, tag="ident")
            mtri = slabs.tile([128, 128], F32R, tag="mtri")  # 1 if j>=k else 0

            # ---- constants
            make_identity(nc, ident[:])
            mtri_f = slabs.tile([128, 128], F32, tag="mtri_f")
            nc.gpsimd.memset(mtri_f[:], 1.0)
            # keep 1.0 where (j - k) >= 0 else 0.0
            nc.gpsimd.affine_select(
                out=mtri_f[:],
                in_=mtri_f[:],
                compare_op=mybir.AluOpType.is_ge,
                fill=0.0,
                base=0,
                pattern=[[1, 128]],
                channel_multiplier=-1,
            )
            nc.vector.tensor_copy(mtri[:], mtri_f[:])
            # ones columns of vN (constant for the whole run)
            ones_f = slabs.tile([128, 64], F32, tag="ones_f")
            nc.gpsimd.memset(ones_f[:], 1.0)
            for ktg in range(NTB * 4):
                nc.vector.tensor_copy(vN[:, ktg, 0, 64:128], ones_f[:])
                nc.vector.tensor_copy(vN[:, ktg, 1, 0:64], ones_f[:])

            # ---- weight loads
            nc.sync.dma_start(wq_s[:], r32(wq_d.rearrange("(ct p) f -> p ct f", p=128)))
            nc.sync.dma_start(wk_s[:], r32(wk_d.rearrange("(ct p) f -> p ct f", p=128)))
            nc.sync.dma_start(wv_s[:], r32(wv_d.rearrange("(ct p) f -> p ct f", p=128)))
            nc.sync.dma_start(wo_s[:], r32(wo_d[:]))

            # ---- phase A: QKV projections (+ V transpose to natural layout)
            for tb in range(NTB):
                xts = []
                for ct in range(CT):
                    xt_t = xtp.tile([128, TBLK], F32R, tag="xt")
                    nc.sync.dma_start(
                        xt_t[:],
                        r32(xt_d[ct * 128 : (ct + 1) * 128, tb * TBLK : (tb + 1) * TBLK]),
                    )
                    xts.append(xt_t)
                for name, w_s, dstT in (("q", wq_s, qT), ("k", wk_s, kT), ("v", wv_s, None)):
                    ps = psA.tile([128, TBLK], F32, tag="ps")
                    for ct in range(CT):
                        nc.tensor.matmul(
                            ps[:],
                            w_s[:, ct, :],
                            xts[ct][:],
                            start=(ct == 0),
                            stop=(ct == CT - 1),
                        )
                    if dstT is not None:
                        nc.vector.tensor_copy(
                            dstT[:, tb * TBLK : (tb + 1) * TBLK], ps[:]
                        )
                    else:
                        vt_t = work.tile([128, TBLK], F32, tag="vt")
                        nc.vector.tensor_copy(vt_t[:], ps[:])
                        # transpose [64d, 128k] -> [128k, 64d] pieces
                        for sub in range(TBLK // 128):
                            ktg = tb * 4 + sub
                            for h in range(HPC):
                                tps = psA.tile([128, 64], F32, tag="ps")
                                nc.tensor.transpose(
                                    tps[:],
                                    vt_t[
                                        h * 64 : (h + 1) * 64,
                                        sub * 128 : (sub + 1) * 128,
                                    ],
                                    ident[h * 64 : (h + 1) * 64, h * 64 : (h + 1) * 64],
                                )
                                vcol = 0 if h == 0 else 64
                                nc.vector.tensor_copy(
                                    vN[:, ktg, h, vcol : vcol + 64], tps[:]
                                )

            # ---- phase B+C: attention per (batch, qblock, head), then out-proj
            for b in range(B):
                for qb in range(QB):
                    t0 = b * T + qb * TBLK  # global token offset of this q block
                    for h in range(HPC):
                        hp = h * 64  # feature partition base for this head
                        O_ps = psO.tile([128, TBLK], F32, tag="O")
                        nkt = (qb + 1) * 4
                        for kt in range(nkt):
                            s = kt * 128 - qb * TBLK  # diag offset, >0 on band
                            col0 = max(s, 0)
                            ncols = TBLK - col0
                            ktg = b * KT + kt
                            sT = psA.tile([128, TBLK], F32, tag="ps")
                            nc.tensor.matmul(
                                sT[:, col0:TBLK],
                                kT[hp : hp + 64, b * T + kt * 128 : b * T + (kt + 1) * 128],
                                qT[hp : hp + 64, t0 + col0 : t0 + TBLK],
                                start=True,
                                stop=True,
                            )
                            es = work.tile([128, TBLK], F32R, tag="es")
                            nc.scalar.activation(
                                es[:, col0:TBLK],
                                sT[:, col0:TBLK],
                                mybir.ActivationFunctionType.Exp,
                                scale=0.125,
                            )
                            if s >= 0:  # diagonal tile: mask strict-lower triangle
                                nc.vector.tensor_mul(
                                    es[:, col0 : col0 + 128],
                                    es[:, col0 : col0 + 128],
                                    mtri[:],
                                )
                            nc.tensor.matmul(
                                O_ps[:, col0:TBLK],
                                vN[:, ktg, h, :],
                                es[:, col0:TBLK],
                                start=(kt == 0),
                                stop=(kt == nkt - 1),
                            )
                        # normalize: O / rowsum
                        rin = work.tile([128, TBLK], F32, tag="rin")
                        if h == 0:
                            nc.vector.reciprocal(rin[64:128, :], O_ps[64:128, :])
                            nc.vector.tensor_mul(
                                oN[0:64, t0 : t0 + TBLK], O_ps[0:64, :], rin[64:128, :]
                            )
                        else:
                            nc.vector.reciprocal(rin[0:64, :], O_ps[0:64, :])
                            nc.vector.tensor_mul(
                                oN[64:128, t0 : t0 + TBLK],
                                O_ps[64:128, :],
                                rin[0:64, :],
                            )
                    # out-projection for this token block
                    for ct in range(CT):
                        ops = psA.tile([128, TBLK], F32, tag="ps")
                        nc.tensor.matmul(
                            ops[:],
                            wo_s[:, ct * 128 : (ct + 1) * 128],
                            oN[:, t0 : t0 + TBLK],
                            start=True,
                            stop=True,
                        )
                        ot = outp.tile([128, TBLK], F32, tag="ot")
                        nc.vector.tensor_copy(ot[:], ops[:])
                        nc.sync.dma_start(
                            out_d[ct * 128 : (ct + 1) * 128, t0 : t0 + TBLK], ot[:]
                        )

    nc.compile()
    return nc


_NC_CACHE = None


def get_program():
    global _NC_CACHE
    if _NC_CACHE is None:
        _NC_CACHE = build_program()
    return _NC_CACHE


def make_in_maps(x, Wq, Wk, Wv, Wo):
    xt = np.ascontiguousarray(x.reshape(NT, C).T)
    in_maps = []
    for cid in range(N_CORES):
        sl = slice(cid * FPC, (cid + 1) * FPC)
        in_maps.append(
            {
                "xt": xt,
                "wq": np.ascontiguousarray(Wq[:, sl]),
                "wk": np.ascontiguousarray(Wk[:, sl]),
                "wv": np.ascontiguousarray(Wv[:, sl]),
                "wo": np.ascontiguousarray(Wo[sl, :]),
            }
        )
    return in_maps


def kernel(x, Wq, Wk, Wv, Wo, bo, _trace=False, _tmpdir=None):
    x = np.asarray(x, dtype=np.float32)
    in_maps = make_in_maps(
        x,
        np.asarray(Wq, np.float32),
        np.asarray(Wk, np.float32),
        np.asarray(Wv, np.float32),
        np.asarray(Wo, np.float32),
    )
    nc = get_program()
    res = run_bass_kernel_spmd(
        nc, in_maps, core_ids=list(range(N_CORES)), trace=_trace, tmpdir=_tmpdir
    )
    acc = res.results[0]["outT"].astype(np.float32)
    for i in range(1, N_CORES):
        acc = acc + res.results[i]["outT"]
    out = acc.T + np.asarray(bo, np.float32)[None, :]
    if _trace:
        kernel._last_results = res
    return out.reshape(B, T, C).astype(np.float32)


# revision 9
# speedup vs baseline: 1.2291x; 1.2291x over previous
"""Multi-head causal attention (B=2, T=2048, H=16, D=64, C=1024) on 8 trn2 cores.

Sharding: tensor-parallel over heads. Each core owns 2 heads (both batches):
  - computes Q^T/K^T/V^T for its heads over all 4096 tokens
  - causal attention in transposed orientation (S^T[k,q]) so no P transpose
  - partial output projection outT_partial[c, t] = Wo_slice^T @ O^T
Host sums the 8 partials (the "all-reduce"), adds bias, transposes back.

All matmuls run as float32r (fp32 storage, full PE rate for moving dim>=256).
"""

import sys

sys.path.insert(0, "/opt/trn_rl_repo")
sys.path.insert(0, "/root/problem")

import numpy as np

import concourse.bacc as bacc
import concourse.mybir as mybir
import concourse.tile as tile
from concourse.bass_utils import run_bass_kernel_spmd
from concourse.masks import make_identity

B, T, C = 2, 2048, 1024
H, D = 16, 64
NT = B * T  # 4096 flattened tokens
N_CORES = 8
HPC = H // N_CORES  # 2 heads per core
FPC = HPC * D  # 128 features per core
CT = C // 128  # 8 contraction tiles for projections
TBLK = 512  # token block
NTB = NT // TBLK  # 8 token blocks
QB = T // TBLK  # 4 query blocks per batch
KT = T // 128  # 16 key tiles per batch

F32 = mybir.dt.float32
F32R = mybir.dt.float32r


def r32(ap):
    return ap.bitcast(F32R)


def build_program():
    nc = bacc.Bacc("TRN2", target_bir_lowering=False, debug=False)

    xt_d = nc.declare_dram_parameter("xt", [C, NT], F32, isOutput=False)
    wq_d = nc.declare_dram_parameter("wq", [C, FPC], F32, isOutput=False)
    wk_d = nc.declare_dram_parameter("wk", [C, FPC], F32, isOutput=False)
    wv_d = nc.declare_dram_parameter("wv", [C, FPC], F32, isOutput=False)
    wo_d = nc.declare_dram_parameter("wo", [FPC, C], F32, isOutput=False)
    out_d = nc.declare_dram_parameter("outT", [C, NT], F32, isOutput=True)

    with tile.TileContext(nc) as tc:
        with (
            tc.tile_pool(name="slabs", bufs=1) as slabs,
            tc.tile_pool(name="xtp", bufs=16) as xtp,
            tc.tile_pool(name="esp", bufs=3) as esp,
            tc.tile_pool(name="vtp", bufs=2) as vtp,
            tc.tile_pool(name="rinp", bufs=2) as rinp,
            tc.tile_pool(name="outp", bufs=4) as outp,
            tc.tile_pool(name="psA", bufs=2, space="PSUM") as psA,
            tc.tile_pool(name="psS", bufs=2, space="PSUM") as psS,
            tc.tile_pool(name="psO", bufs=2, space="PSUM") as psO,
        ):
            # ---- persistent slabs
            qT = slabs.tile([128, NT], F32R, tag="qT")  # [2h*64d, t]
            kT = slabs.tile([128, NT], F32R, tag="kT")
            # V natural layout + ones cols: per (ktile_global, h): [128k, 128]
            # h=0: cols 0:64 = V, 64:128 = ones   (O rows 0:64, rowsum 64:128)
            # h=1: cols 0:64 = ones, 64:128 = V   (rowsum rows 0:64, O 64:128)
            vN = slabs.tile([128, NTB * 4, HPC, 128], F32R, tag="vN")
            oN = slabs.tile([128, NT], F32R, tag="oN")  # normalized O^T
            wq_s = slabs.tile([128, CT, FPC], F32R, tag="wq")
            wk_s = slabs.tile([128, CT, FPC], F32R, tag="wk")
            wv_s = slabs.tile([128, CT, FPC], F32R, tag="wv")
            wo_s = slabs.tile([128, C], F32R, tag="wo")  # [f, c]
            ident = slabs.tile([128, 128], F32, tag="ident")
            mtri = slabs.tile([128, 128], F32R, tag="mtri")  # 1 if j>=k else 0

            # ---- constants
            make_identity(nc, ident[:])
            mtri_f = slabs.tile([128, 128], F32, tag="mtri_f")
            nc.gpsimd.memset(mtri_f[:], 1.0)
            # keep 1.0 where (j - k) >= 0 else 0.0
            nc.gpsimd.affine_select(
                out=mtri_f[:],
                in_=mtri_f[:],
                compare_op=mybir.AluOpType.is_ge,
                fill=0.0,
                base=0,
                pattern=[[1, 128]],
                channel_multiplier=-1,
            )
            nc.vector.tensor_copy(mtri[:], mtri_f[:])
            # ones columns of vN (constant for the whole run)
            ones_f = slabs.tile([128, 64], F32, tag="ones_f")
            nc.gpsimd.memset(ones_f[:], 1.0)
            for ktg in range(NTB * 4):
                nc.vector.tensor_copy(vN[:, ktg, 0, 64:128], ones_f[:])
                nc.vector.tensor_copy(vN[:, ktg, 1, 0:64], ones_f[:])

            # ---- weight loads
            nc.sync.dma_start(wq_s[:], r32(wq_d.rearrange("(ct p) f -> p ct f", p=128)))
            nc.sync.dma_start(wk_s[:], r32(wk_d.rearrange("(ct p) f -> p ct f", p=128)))
            nc.sync.dma_start(wv_s[:], r32(wv_d.rearrange("(ct p) f -> p ct f", p=128)))
            nc.sync.dma_start(wo_s[:], r32(wo_d[:]))

            # ---- phase A: QKV projections (+ V transpose to natural layout)
            for tb in range(NTB):
                xts = []
                for ct in range(CT):
                    xt_t = xtp.tile([128, TBLK], F32R, tag="xt")
                    nc.sync.dma_start(
                        xt_t[:],
                        r32(xt_d[ct * 128 : (ct + 1) * 128, tb * TBLK : (tb + 1) * TBLK]),
                    )
                    xts.append(xt_t)
                for name, w_s, dstT in (("q", wq_s, qT), ("k", wk_s, kT), ("v", wv_s, None)):
                    ps = psA.tile([128, TBLK], F32, tag="ps")
                    for ct in range(CT):
                        nc.tensor.matmul(
                            ps[:],
                            w_s[:, ct, :],
                            xts[ct][:],
                            start=(ct == 0),
                            stop=(ct == CT - 1),
                        )
                    if dstT is not None:
                        nc.vector.tensor_copy(
                            dstT[:, tb * TBLK : (tb + 1) * TBLK], ps[:]
                        )
                    else:
                        vt_t = vtp.tile([128, TBLK], F32, tag="vt")
                        nc.vector.tensor_copy(vt_t[:], ps[:])
                        # transpose [64d, 128k] -> [128k, 64d] pieces
                        for sub in range(TBLK // 128):
                            ktg = tb * 4 + sub
                            for h in range(HPC):
                                tps = psA.tile([128, 64], F32, tag="ps")
                                nc.tensor.transpose(
                                    tps[:],
                                    vt_t[
                                        h * 64 : (h + 1) * 64,
                                        sub * 128 : (sub + 1) * 128,
                                    ],
                                    ident[h * 64 : (h + 1) * 64, h * 64 : (h + 1) * 64],
                                )
                                vcol = 0 if h == 0 else 64
                                nc.vector.tensor_copy(
                                    vN[:, ktg, h, vcol : vcol + 64], tps[:]
                                )

            # ---- phase B+C: attention per (batch, qblock), heads interleaved
            for b in range(B):
                for qb in range(QB):
                    t0 = b * T + qb * TBLK  # global token offset of this q block
                    O_ps = [psO.tile([128, TBLK], F32, tag="O", name=f"O_{b}_{qb}_{h}") for h in range(HPC)]
                    nkt = (qb + 1) * 4
                    for kt in range(nkt):
                        s = kt * 128 - qb * TBLK  # diag offset, >=0 on band
                        col0 = max(s, 0)
                        ktg = b * KT + kt
                        sT = psS.tile([128, HPC, TBLK], F32, tag="sT")
                        es = esp.tile([128, HPC, TBLK], F32R, tag="es")
                        for h in range(HPC):
                            hp = h * 64
                            nc.tensor.matmul(
                                sT[:, h, col0:TBLK],
                                kT[hp : hp + 64, b * T + kt * 128 : b * T + (kt + 1) * 128],
                                qT[hp : hp + 64, t0 + col0 : t0 + TBLK],
                                start=True,
                                stop=True,
                            )
                        if False:
                            pass
                        else:
                            for h in range(HPC):
                                nc.scalar.activation(
                                    es[:, h, col0:TBLK],
                                    sT[:, h, col0:TBLK],
                                    mybir.ActivationFunctionType.Exp,
                                    scale=0.125,
                                )
                        if s >= 0:  # diagonal tile: mask strict-lower triangle
                            for h in range(HPC):
                                nc.vector.tensor_mul(
                                    es[:, h, col0 : col0 + 128],
                                    es[:, h, col0 : col0 + 128],
                                    mtri[:],
                                )
                        for h in range(HPC):
                            nc.tensor.matmul(
                                O_ps[h][:, col0:TBLK],
                                vN[:, ktg, h, :],
                                es[:, h, col0:TBLK],
                                start=(kt == 0),
                                stop=(kt == nkt - 1),
                            )
                    # normalize: O / rowsum (rowsum rows: h0 -> 64:128, h1 -> 0:64)
                    rin = rinp.tile([128, TBLK], F32, tag="rin")
                    nc.vector.reciprocal(rin[64:128, :], O_ps[0][64:128, :])
                    nc.vector.tensor_mul(
                        oN[0:64, t0 : t0 + TBLK], O_ps[0][0:64, :], rin[64:128, :]
                    )
                    nc.vector.reciprocal(rin[0:64, :], O_ps[1][0:64, :])
                    nc.vector.tensor_mul(
                        oN[64:128, t0 : t0 + TBLK], O_ps[1][64:128, :], rin[0:64, :]
                    )
                    # out-projection for this token block
                    for ct in range(CT):
                        ops = psA.tile([128, TBLK], F32, tag="ps")
                        nc.tensor.matmul(
                            ops[:],
                            wo_s[:, ct * 128 : (ct + 1) * 128],
                            oN[:, t0 : t0 + TBLK],
                            start=True,
                            stop=True,
                        )
                        ot = outp.tile([128, TBLK], F32, tag="ot")
                        nc.vector.tensor_copy(ot[:], ops[:])
                        nc.sync.dma_start(
                            out_d[ct * 128 : (ct + 1) * 128, t0 : t0 + TBLK], ot[:]
                        )

    nc.compile()
    return nc


_NC_CACHE = None


def get_program():
    global _NC_CACHE
    if _NC_CACHE is None:
        _NC_CACHE = build_program()
    return _NC_CACHE


def make_in_maps(x, Wq, Wk, Wv, Wo):
    xt = np.ascontiguousarray(x.reshape(NT, C).T)
    in_maps = []
    for cid in range(N_CORES):
        sl = slice(cid * FPC, (cid + 1) * FPC)
        in_maps.append(
            {
                "xt": xt,
                "wq": np.ascontiguousarray(Wq[:, sl]),
                "wk": np.ascontiguousarray(Wk[:, sl]),
                "wv": np.ascontiguousarray(Wv[:, sl]),
                "wo": np.ascontiguousarray(Wo[sl, :]),
            }
        )
    return in_maps


def kernel(x, Wq, Wk, Wv, Wo, bo, _trace=False, _tmpdir=None):
    x = np.asarray(x, dtype=np.float32)
    in_maps = make_in_maps(
        x,
        np.asarray(Wq, np.float32),
        np.asarray(Wk, np.float32),
        np.asarray(Wv, np.float32),
        np.asarray(Wo, np.float32),
    )
    nc = get_program()
    res = run_bass_kernel_spmd(
        nc, in_maps, core_ids=list(range(N_CORES)), trace=_trace, tmpdir=_tmpdir
    )
    acc = res.results[0]["outT"].astype(np.float32)
    for i in range(1, N_CORES):
        acc = acc + res.results[i]["outT"]
    out = acc.T + np.asarray(bo, np.float32)[None, :]
    if _trace:
        kernel._last_results = res
    return out.reshape(B, T, C).astype(np.float32)
